# revision 5
# baseline (speedup 1.0000x reference)
"""DropStripes Trainium2 kernel.

out[b, t, f] = x[b, t, f] * keep[b, f], where keep[b, f] = 0 iff f falls in
any stripe [bgn[b,s], bgn[b,s]+distance[b,s]) for s in range(STRIPES).

Strategy: pure data-parallel over the batch dim (64 batches -> 8 cores x 8
batches each). The (B, F) keep mask is expanded from the tiny (B, S) index
arrays on the host; each core streams its batches through SBUF in units of
(125 partitions x kn rows x 512): SWDGE load -> in-place DVE tensor_tensor
multiply against the batch's mask row (stride-0 broadcast across the row
dim) -> SWDGE store.

Memory-bound, so the payload dtype is bf16: the host rounds x f32->bf16
(max rel err ~2^-9 = 0.2%, far inside the 2e-2 gate; the 0/1 mask is exact
in bf16 so the multiply adds no further error), the device moves/multiplies
bf16, and the host widens the result back to f32. Per-core traffic is
~16.5 MB read + 16 MB write; in the mixed read+write steady state the 16
SDMA engines sustain ~380 GB/s aggregate (near the 8-core chip HBM
roofline), while pure-read phases run read packets at ~half rate (~230
GB/s) — so the schedule's job is to keep stores flowing as early and as
continuously as possible. Each SWDGE DMA is served by a rotating window of
5 of the 16 SDMA engines (25 descriptors each; window of DMA i is engines
{(5i+k) mod 16}), so >=4 DMAs always in flight keep all engines fed. The
first batch is split into small units (kn=2,2,4,8) so the first store
enters the rings ~15 us earlier than with uniform 1 MB units.
"""

import sys

if "/opt/trn_rl_repo" not in sys.path:
    sys.path.insert(0, "/opt/trn_rl_repo")

import numpy as np

B, T, F = 64, 2000, 512
N_CORES = 8
BPC = B // N_CORES  # batches per core
P = 125  # SBUF partitions used (125 * 16 = 2000 rows)
K = T // P  # rows of F per partition
KN_MAX = 8  # steady-state rows per work unit: 125p x 8 x 512 x 2B = 1 MB DMAs
HEAD_KNS = [8, 8]  # batch-0 units (uniform: head ramp experiments regressed)
PF = 6
NBUF = 16  # SBUF slots; >= PF+2 so slot-WAR waits are long satisfied

_cached = {}


def _demote_deps(bass_ins, keep_names):
    """Keep only `keep_names` as semaphore-wait (sync) deps; demote the rest
    to nosync (scheduler-ordering-only) deps.

    Tile's sem pass is not transitively minimal: the multiply would wait on
    its load, on the store that freed its SBUF slot (already implied by the
    load's own WAR wait), and on an earlier same-engine DVE op (implied by
    in-order execution). Demotion preserves scheduler ordering, so the
    implication chains stay valid.
    """
    from concourse.instruction_name_ordered_set import InstructionNameOrderedSet

    ins = bass_ins.ins
    cur = ins.sync_dependency_set_copy()
    keep = InstructionNameOrderedSet([n for n in cur if n in keep_names])
    demote = cur.difference(keep)
    ins.set_sync_dependencies(keep)
    ins.add_nosync_dependencies_from(demote)



_birsim_patched = False


def _patch_birsim():
    """Disable the BIR simulator pass in walrus: it rejects multi-wait
    instructions that the real codegen handles."""
    global _birsim_patched
    if _birsim_patched:
        return
    import concourse.bass_utils as bu

    orig = bu.run_command

    def patched(argv, **kwargs):
        argv = [
            a.replace("--enable-birsim=true", "--enable-birsim=false") for a in argv
        ]
        return orig(argv, **kwargs)

    bu.run_command = patched
    _birsim_patched = True


def _build_program():
    _patch_birsim()
    import concourse.bass as bass
    import concourse.mybir as mybir
    from concourse.tile import TileContext

    DT = mybir.dt.bfloat16
    nc = bass.Bass()

    x = nc.dram_tensor("x", [BPC, T, F], DT, kind="ExternalInput")
    # Host pre-replicates each batch's keep-mask row across the 125 SBUF
    # partitions as uint8 (0/1): mask[p, b*F + f] = keep[b, f]. SWDGE casts
    # it to bf16 during the (single, upfront) DMA, so mask read traffic is
    # 512 KB instead of 1 MB.
    mask = nc.dram_tensor("mask", [P, BPC * F], mybir.dt.uint8, kind="ExternalInput")
    out = nc.dram_tensor("out", [BPC, T, F], DT, kind="ExternalOutput")

    # All bulk DMAs go through SWDGE (gpsimd): this runtime fans one HWDGE
    # DMA over only 5 fixed SDMA engines, while SWDGE round-robins
    # descriptors across all 16 (5-engine rotating window per DMA).
    # Everything issues from the single POOL engine, so the loop is
    # software-pipelined by hand: upcoming loads are issued BEFORE
    # store(i), and the store's wait on the multiply therefore never
    # stalls them.
    # Work units: (batch, row_start, n_rows) in K-rows-per-partition terms.
    units = []
    k0 = 0
    for kn in HEAD_KNS:
        units.append((0, k0, kn))
        k0 += kn
    assert k0 == K
    for b in range(1, BPC):
        for k0 in range(0, K, KN_MAX):
            units.append((b, k0, KN_MAX))
    assert NBUF == len(units)
    loads, tts, stores, mask_lds = [], [], [], []

    def _mk_load(i, tiles, xp):
        from concourse.instruction_name_ordered_set import (
            InstructionNameOrderedSet,
        )

        b, k0, kn = units[i]
        # Uniform-size slots keep the pool allocation trivial; small head
        # units use a prefix slice.
        t = xp.tile([P, KN_MAX * F], DT)
        src = x[b].rearrange("(p k) f -> p k f", p=P)[:, k0 : k0 + kn, :]
        ld = nc.gpsimd.dma_start(out=t[:, : kn * F], in_=src)
        ld_keep = {stores[i - NBUF].ins.name} if i >= NBUF else set()
        _demote_deps(ld, ld_keep)
        # Ordering-only edge: the scheduler must keep the upfront mask DMA
        # ahead of every load in the POOL stream.
        ld.ins.add_nosync_dependencies_from(
            InstructionNameOrderedSet([mask_lds[0].ins.name])
        )
        loads.append(ld)
        tiles[i] = t

    with TileContext(nc) as tc:
        with (
            tc.tile_pool(name="xp", bufs=NBUF) as xp,
            tc.tile_pool(name="mp", bufs=1) as mp,
        ):
            m = mp.tile([P, BPC * F], DT)
            mld = nc.gpsimd.dma_start(out=m[:], in_=mask[:])
            _demote_deps(mld, set())
            mask_lds.append(mld)
            tiles = {}
            for i in range(min(PF, len(units))):
                _mk_load(i, tiles, xp)
            for i, (b, k0, kn) in enumerate(units):
                if i + PF < len(units):
                    _mk_load(i + PF, tiles, xp)
                t = tiles.pop(i)
                t3 = t[:, : kn * F].rearrange("p (k f) -> p k f", f=F)
                mb = m[:, b * F : (b + 1) * F]
                tt = nc.vector.tensor_tensor(
                    out=t3,
                    in0=t3,
                    in1=mb[:, None, :].to_broadcast((P, kn, F)),
                    op=mybir.AluOpType.mult,
                )
                # The first multiply must semaphore-wait the mask DMA (its
                # window of SDMA engines is disjoint from the loads', so
                # load completion does not imply mask residency). Later
                # multiplies are covered by DVE in-order execution.
                tt_keep = {loads[i].ins.name}
                if i == 0:
                    tt_keep.add(mld.ins.name)
                _demote_deps(tt, tt_keep)

                dst = out[b].rearrange("(p k) f -> p k f", p=P)[:, k0 : k0 + kn, :]
                st = nc.gpsimd.dma_start(out=dst, in_=t[:, : kn * F])
                _demote_deps(st, {tt.ins.name})
                tts.append(tt)
                stores.append(st)

    # This walrus build accepts only ONE sync wait per instruction
    # ("Too many sync wait commands"), while Tile freely emits several.
    # Universal fix: for any instruction with k>1 waits, keep the last and
    # hoist the others onto standalone EventSemaphore carriers inserted
    # just before it in the same engine stream. Sequencers execute in
    # order, so the blocking semantics are exactly Tile's.
    for bb in nc.main_func.blocks:
        newlist = []
        n_split = 0
        for ins in bb.instructions:
            si = ins.sync_info
            if si is not None and len(si.on_wait) > 1:
                for w in si.on_wait[:-1]:
                    n_split += 1
                    newlist.append(
                        mybir.InstEventSemaphore(
                            name=f"{ins.name}_wsplit{n_split}",
                            engine=ins.engine,
                            sync_info=mybir.SyncInfo(on_wait=[w], on_update=[]),
                        )
                    )
                ins.sync_info = mybir.SyncInfo(
                    on_wait=[si.on_wait[-1]], on_update=si.on_update
                )
            newlist.append(ins)
        bb.instructions = newlist
    return nc


def _expand_mask(bgn: np.ndarray, distance: np.ndarray) -> np.ndarray:
    pos = np.arange(F)
    bgn = np.asarray(bgn).astype(np.int64)
    dist = np.asarray(distance).astype(np.int64)
    in_stripe = (pos[None, None, :] >= bgn[:, :, None]) & (
        pos[None, None, :] < (bgn + dist)[:, :, None]
    )
    keep = ~np.any(in_stripe, axis=1)  # (B, F)
    return keep.astype(np.uint8)


def kernel(x, bgn, distance, _trace=False, _trace_kwargs=None):
    import ml_dtypes

    from concourse.bass_utils import run_bass_kernel_spmd

    bf16 = ml_dtypes.bfloat16
    x = np.ascontiguousarray(np.asarray(x, dtype=np.float32).astype(bf16))
    keep = _expand_mask(bgn, distance)

    if "nc" not in _cached:
        _cached["nc"] = _build_program()
    nc = _cached["nc"]

    in_maps = []
    for i in range(N_CORES):
        sl = slice(i * BPC, (i + 1) * BPC)
        # (BPC, F) -> (P, BPC*F): each partition row holds all BPC mask rows.
        mask_rep = np.ascontiguousarray(
            np.broadcast_to(keep[sl].reshape(1, BPC * F), (P, BPC * F))
        )
        in_maps.append({"x": x[sl], "mask": mask_rep})

    res = run_bass_kernel_spmd(
        nc, in_maps, list(range(N_CORES)), trace=_trace, **(_trace_kwargs or {})
    )
    _cached["last_results"] = res
    return np.concatenate(
        [r["out"].astype(np.float32) for r in res.results], axis=0
    )


# revision 6
# speedup vs baseline: 1.1269x; 1.1269x over previous
"""DropStripes Trainium2 kernel.

out[b, t, f] = x[b, t, f] * keep[b, f], where keep[b, f] = 0 iff f falls in
any stripe [bgn[b,s], bgn[b,s]+distance[b,s]) for s in range(STRIPES).

Strategy: pure data-parallel over the batch dim (64 batches -> 8 cores x 8
batches each). The (B, F) keep mask is expanded from the tiny (B, S) index
arrays on the host; each core streams its batches through SBUF in units of
(125 partitions x kn rows x 512): SWDGE load -> in-place DVE tensor_tensor
multiply against the batch's mask row (stride-0 broadcast across the row
dim) -> SWDGE store.

Memory-bound, so the payload dtype is bf16: the host rounds x f32->bf16
(max rel err ~2^-9 = 0.2%, far inside the 2e-2 gate; the 0/1 mask is exact
in bf16 so the multiply adds no further error), the device moves/multiplies
bf16, and the host widens the result back to f32. Per-core traffic is
~16.5 MB read + 16 MB write; in the mixed read+write steady state the 16
SDMA engines sustain ~380 GB/s aggregate (near the 8-core chip HBM
roofline), while pure-read phases run read packets at ~half rate (~230
GB/s) — so the schedule's job is to keep stores flowing as early and as
continuously as possible. Each SWDGE DMA is served by a rotating window of
5 of the 16 SDMA engines (25 descriptors each; window of DMA i is engines
{(5i+k) mod 16}), so >=4 DMAs always in flight keep all engines fed. The
first batch is split into small units (kn=2,2,4,8) so the first store
enters the rings ~15 us earlier than with uniform 1 MB units.
"""

import sys

if "/opt/trn_rl_repo" not in sys.path:
    sys.path.insert(0, "/opt/trn_rl_repo")

import numpy as np

B, T, F = 64, 2000, 512
N_CORES = 8
BPC = B // N_CORES  # batches per core
P = 125  # SBUF partitions used (125 * 16 = 2000 rows)
K = T // P  # rows of F per partition
KN_MAX = 8  # steady-state rows per work unit: 125p x 8 x 512 x 2B = 1 MB DMAs
HEAD_KNS = [8, 8]  # batch-0 units (uniform: head ramp experiments regressed)
PF = 3
NBUF = 16  # SBUF slots; >= PF+2 so slot-WAR waits are long satisfied

_cached = {}


def _demote_deps(bass_ins, keep_names):
    """Keep only `keep_names` as semaphore-wait (sync) deps; demote the rest
    to nosync (scheduler-ordering-only) deps.

    Tile's sem pass is not transitively minimal: the multiply would wait on
    its load, on the store that freed its SBUF slot (already implied by the
    load's own WAR wait), and on an earlier same-engine DVE op (implied by
    in-order execution). Demotion preserves scheduler ordering, so the
    implication chains stay valid.
    """
    from concourse.instruction_name_ordered_set import InstructionNameOrderedSet

    ins = bass_ins.ins
    cur = ins.sync_dependency_set_copy()
    keep = InstructionNameOrderedSet([n for n in cur if n in keep_names])
    demote = cur.difference(keep)
    ins.set_sync_dependencies(keep)
    ins.add_nosync_dependencies_from(demote)



_birsim_patched = False


def _patch_birsim():
    """Disable the BIR simulator pass in walrus: it rejects multi-wait
    instructions that the real codegen handles."""
    global _birsim_patched
    if _birsim_patched:
        return
    import concourse.bass_utils as bu

    orig = bu.run_command

    def patched(argv, **kwargs):
        argv = [
            a.replace("--enable-birsim=true", "--enable-birsim=false") for a in argv
        ]
        return orig(argv, **kwargs)

    bu.run_command = patched
    _birsim_patched = True


def _build_program():
    _patch_birsim()
    import concourse.bass as bass
    import concourse.mybir as mybir
    from concourse.tile import TileContext

    DT = mybir.dt.bfloat16
    nc = bass.Bass()

    x = nc.dram_tensor("x", [BPC, T, F], DT, kind="ExternalInput")
    # Host pre-replicates each batch's keep-mask row across the 125 SBUF
    # partitions as uint8 (0/1): mask[p, b*F + f] = keep[b, f]. SWDGE casts
    # it to bf16 during the (single, upfront) DMA, so mask read traffic is
    # 512 KB instead of 1 MB.
    mask = nc.dram_tensor("mask", [P, BPC * F], mybir.dt.uint8, kind="ExternalInput")
    out = nc.dram_tensor("out", [BPC, T, F], DT, kind="ExternalOutput")

    # All bulk DMAs go through SWDGE (gpsimd): this runtime fans one HWDGE
    # DMA over only 5 fixed SDMA engines, while SWDGE round-robins
    # descriptors across all 16 (5-engine rotating window per DMA).
    # Everything issues from the single POOL engine, so the loop is
    # software-pipelined by hand: upcoming loads are issued BEFORE
    # store(i), and the store's wait on the multiply therefore never
    # stalls them.
    # Work units: (batch, row_start, n_rows) in K-rows-per-partition terms.
    units = []
    k0 = 0
    for kn in HEAD_KNS:
        units.append((0, k0, kn))
        k0 += kn
    assert k0 == K
    for b in range(1, BPC):
        for k0 in range(0, K, KN_MAX):
            units.append((b, k0, KN_MAX))
    assert NBUF == len(units)
    loads, tts, stores, mask_lds = [], [], [], []

    def _mk_load(i, tiles, xp):
        from concourse.instruction_name_ordered_set import (
            InstructionNameOrderedSet,
        )

        b, k0, kn = units[i]
        # Uniform-size slots keep the pool allocation trivial; small head
        # units use a prefix slice.
        t = xp.tile([P, KN_MAX * F], DT)
        src = x[b].rearrange("(p k) f -> p k f", p=P)[:, k0 : k0 + kn, :]
        ld = nc.gpsimd.dma_start(out=t[:, : kn * F], in_=src)
        ld_keep = {stores[i - NBUF].ins.name} if i >= NBUF else set()
        _demote_deps(ld, ld_keep)
        # Ordering-only edge: the scheduler must keep the upfront mask DMA
        # ahead of every load in the POOL stream.
        ld.ins.add_nosync_dependencies_from(
            InstructionNameOrderedSet([mask_lds[0].ins.name])
        )
        loads.append(ld)
        tiles[i] = t

    with TileContext(nc) as tc:
        with (
            tc.tile_pool(name="xp", bufs=NBUF) as xp,
            tc.tile_pool(name="mp", bufs=1) as mp,
        ):
            m = mp.tile([P, BPC * F], DT)
            mld = nc.gpsimd.dma_start(out=m[:], in_=mask[:])
            _demote_deps(mld, set())
            mask_lds.append(mld)
            tiles = {}
            for i in range(min(PF, len(units))):
                _mk_load(i, tiles, xp)
            for i, (b, k0, kn) in enumerate(units):
                if i + PF < len(units):
                    _mk_load(i + PF, tiles, xp)
                t = tiles.pop(i)
                t3 = t[:, : kn * F].rearrange("p (k f) -> p k f", f=F)
                mb = m[:, b * F : (b + 1) * F]
                tt = nc.vector.tensor_tensor(
                    out=t3,
                    in0=t3,
                    in1=mb[:, None, :].to_broadcast((P, kn, F)),
                    op=mybir.AluOpType.mult,
                )
                # The first multiply must semaphore-wait the mask DMA (its
                # window of SDMA engines is disjoint from the loads', so
                # load completion does not imply mask residency). Later
                # multiplies are covered by DVE in-order execution.
                tt_keep = {loads[i].ins.name}
                if i == 0:
                    tt_keep.add(mld.ins.name)
                _demote_deps(tt, tt_keep)

                dst = out[b].rearrange("(p k) f -> p k f", p=P)[:, k0 : k0 + kn, :]
                st = nc.gpsimd.dma_start(out=dst, in_=t[:, : kn * F])
                _demote_deps(st, {tt.ins.name})
                tts.append(tt)
                stores.append(st)

    # This walrus build accepts only ONE sync wait per instruction
    # ("Too many sync wait commands"), while Tile freely emits several.
    # Universal fix: for any instruction with k>1 waits, keep the last and
    # hoist the others onto standalone EventSemaphore carriers inserted
    # just before it in the same engine stream. Sequencers execute in
    # order, so the blocking semantics are exactly Tile's.
    for bb in nc.main_func.blocks:
        newlist = []
        n_split = 0
        for ins in bb.instructions:
            si = ins.sync_info
            if si is not None and len(si.on_wait) > 1:
                for w in si.on_wait[:-1]:
                    n_split += 1
                    newlist.append(
                        mybir.InstEventSemaphore(
                            name=f"{ins.name}_wsplit{n_split}",
                            engine=ins.engine,
                            sync_info=mybir.SyncInfo(on_wait=[w], on_update=[]),
                        )
                    )
                ins.sync_info = mybir.SyncInfo(
                    on_wait=[si.on_wait[-1]], on_update=si.on_update
                )
            newlist.append(ins)
        bb.instructions = newlist
    return nc


def _expand_mask(bgn: np.ndarray, distance: np.ndarray) -> np.ndarray:
    pos = np.arange(F)
    bgn = np.asarray(bgn).astype(np.int64)
    dist = np.asarray(distance).astype(np.int64)
    in_stripe = (pos[None, None, :] >= bgn[:, :, None]) & (
        pos[None, None, :] < (bgn + dist)[:, :, None]
    )
    keep = ~np.any(in_stripe, axis=1)  # (B, F)
    return keep.astype(np.uint8)


def kernel(x, bgn, distance, _trace=False, _trace_kwargs=None):
    import ml_dtypes

    from concourse.bass_utils import run_bass_kernel_spmd

    bf16 = ml_dtypes.bfloat16
    x = np.ascontiguousarray(np.asarray(x, dtype=np.float32).astype(bf16))
    keep = _expand_mask(bgn, distance)

    if "nc" not in _cached:
        _cached["nc"] = _build_program()
    nc = _cached["nc"]

    in_maps = []
    for i in range(N_CORES):
        sl = slice(i * BPC, (i + 1) * BPC)
        # (BPC, F) -> (P, BPC*F): each partition row holds all BPC mask rows.
        mask_rep = np.ascontiguousarray(
            np.broadcast_to(keep[sl].reshape(1, BPC * F), (P, BPC * F))
        )
        in_maps.append({"x": x[sl], "mask": mask_rep})

    res = run_bass_kernel_spmd(
        nc, in_maps, list(range(N_CORES)), trace=_trace, **(_trace_kwargs or {})
    )
    _cached["last_results"] = res
    return np.concatenate(
        [r["out"].astype(np.float32) for r in res.results], axis=0
    )
